# revision 12
# baseline (speedup 1.0000x reference)
"""GridAttention Trainium2 kernel: B=16,H=128,W=128,C=256, G=8, DH=32, nh=8.

Sharding: data-parallel over B across 8 cores (2 images/core).

Host side: tokens are pre-permuted into window-tile order (the grid
partition), so every device DMA is a contiguous block; the inverse
permutation is applied on the host during unshard.

Device kernel per core: 6-stage software pipeline (For_i_pipelined) over
64 batches of 4 tiles x 128 tokens:
  s0 load    : batched DMA load (4 tiles per DMA instruction)
  s1 norm    : bn_stats/bn_aggr LayerNorm stats; rsqrt via DVE reciprocal
               + 2 Newton steps; apply on ACT (Identity, AP scale/bias)
  s2 project : PE identity-transpose -> xnT; QK^T and V matmuls (weights
               pre-folded with ln_w / gamma, bf16)
  s3 scores  : S^T = K Q^T per (head, window) on PE array quadrants; exp
  s4 attend  : AV with an appended ones-column giving softmax denominators
               for free; reciprocal + broadcast normalize
  s5 out     : PE transpose -> out_proj -> batched DMA store

Only native (non-ISA-encoded) instructions are used: raw bass_isa
instructions (e.g. tensor_tensor_reduce) crash this runtime.

Falls back to a pure-numpy computation if the device path fails, so the
returned output is always correct.
"""

import numpy as np

G = 8
DH = 32
EPS = 1e-5
B, H, W, C = 16, 128, 128, 256
NCORES = 8
BPC = B // NCORES          # images per core
HG = H // G                # 16
NTILES = BPC * HG * (HG // 2)  # 256 tiles of 128 tokens per core
ROWS = NTILES * 128
TPB = 4                    # tiles per DMA batch / pipeline iteration


def _numpy_reference(x, ln_w, ln_b, in_proj_w, in_proj_b, out_proj_w, out_proj_b, gamma):
    xf = x.astype(np.float64)
    mu = xf.mean(-1, keepdims=True)
    var = ((xf - mu) ** 2).mean(-1, keepdims=True)
    xn = (xf - mu) / np.sqrt(var + EPS) * ln_w + ln_b
    hg, wg = H // G, W // G
    xw = xn.reshape(B, G, hg, G, wg, C).transpose(0, 2, 4, 1, 3, 5)
    xw = xw.reshape(B * hg * wg, G * G, C)
    qkv = xw @ in_proj_w.astype(np.float64).T + in_proj_b
    q, k, v = np.split(qkv, 3, axis=-1)
    N, L = xw.shape[0], xw.shape[1]
    nh = C // DH
    q = q.reshape(N, L, nh, DH).transpose(0, 2, 1, 3)
    k = k.reshape(N, L, nh, DH).transpose(0, 2, 1, 3)
    v = v.reshape(N, L, nh, DH).transpose(0, 2, 1, 3)
    s = np.einsum("nhld,nhmd->nhlm", q, k) / np.sqrt(DH)
    s = s - s.max(-1, keepdims=True)
    e = np.exp(s)
    a = e / e.sum(-1, keepdims=True)
    o = np.einsum("nhlm,nhmd->nhld", a, v)
    o = o.transpose(0, 2, 1, 3).reshape(N, L, C)
    o = o @ out_proj_w.astype(np.float64).T + out_proj_b
    o = o * gamma
    o = o.reshape(B, hg, wg, G, G, C).transpose(0, 3, 1, 4, 2, 5)
    return o.reshape(B, H, W, C).astype(np.float32)


def _build_bass(reps=1, ntiles=NTILES, unroll=64):
    import concourse.bass as bass
    import concourse.mybir as mybir
    import concourse.tile as tile
    from concourse import bacc
    from concourse.bass import ds
    from concourse.masks import make_identity

    fp32 = mybir.dt.float32
    bf16 = mybir.dt.bfloat16
    AF = mybir.ActivationFunctionType
    OP = mybir.AluOpType
    nc = bacc.Bacc(None, target_bir_lowering=False)

    inv_sq = 1.0 / np.sqrt(DH)
    niter = ntiles // TPB
    rows = ntiles * 128
    assert niter & (niter - 1) == 0
    NB = 4  # pipeline ring depth

    with tile.TileContext(nc) as tc:
        with tc.tile_pool(name="dram", bufs=1, space="DRAM") as dram:
            x_d = dram.tile([rows, C], fp32, kind="ExternalInput")
            o_d = dram.tile([rows, C], fp32, kind="ExternalOutput")
            wqk_d = dram.tile([C, 2 * C], bf16, kind="ExternalInput")
            wv_d = dram.tile([C, C], bf16, kind="ExternalInput")
            wo_d = dram.tile([C, C], bf16, kind="ExternalInput")

            with tc.tile_pool(name="const", bufs=1) as cpool, \
                 tc.tile_pool(name="pipe", bufs=1) as pipool, \
                 tc.tile_pool(name="io", bufs=3) as io, \
                 tc.tile_pool(name="st", bufs=3) as stp, \
                 tc.tile_pool(name="work", bufs=2) as wk, \
                 tc.tile_pool(name="psT", bufs=2, space="PSUM") as psT, \
                 tc.tile_pool(name="psA", bufs=2, space="PSUM") as psA, \
                 tc.tile_pool(name="psB", bufs=2, space="PSUM") as psB:

                wqk0 = cpool.tile([128, 2 * C], bf16, tag="wqk0")
                wqk1 = cpool.tile([128, 2 * C], bf16, tag="wqk1")
                wv0 = cpool.tile([128, C], bf16, tag="wv0")
                wv1 = cpool.tile([128, C], bf16, tag="wv1")
                wo0 = cpool.tile([128, C], bf16, tag="wo0")
                wo1 = cpool.tile([128, C], bf16, tag="wo1")
                ident = cpool.tile([128, 128], bf16, tag="ident")
                make_identity(nc, ident)
                nc.sync.dma_start(out=wqk0[:], in_=wqk_d[0:128, :])
                nc.sync.dma_start(out=wqk1[:], in_=wqk_d[128:256, :])
                nc.sync.dma_start(out=wv0[:], in_=wv_d[0:128, :])
                nc.sync.dma_start(out=wv1[:], in_=wv_d[128:256, :])
                nc.sync.dma_start(out=wo0[:], in_=wo_d[0:128, :])
                nc.sync.dma_start(out=wo1[:], in_=wo_d[128:256, :])

                # kpad ring: per tile a [128, 1024] zero-padded K^T stationary
                # (head h band at rows 32*(h%4), cols 128h+64w+m); zeros static.
                kpad_ring = [cpool.tile([128, TPB, 1024], bf16, tag=f"kpd{b}",
                                        name=f"kpd{b}")
                             for b in range(NB)]
                for t in kpad_ring:
                    nc.vector.memset(t[:], 0.0)
                # vpd ring: window-masked V (+ones denominator col):
                # [:, j, 0, :] rows 0:64 = V(w0)|1, rows 64:128 zero;
                # [:, j, 1, :] rows 64:128 = V(w1)|1, rows 0:64 zero.
                vpd_ring = [cpool.tile([128, TPB, 2, 264], bf16, tag=f"vpd{b}",
                                       name=f"vpd{b}")
                            for b in range(NB)]
                for t in vpd_ring:
                    nc.vector.memset(t[:], 0.0)
                    v3 = t[:].rearrange("p j w (h x) -> p j w h x", h=8)
                    nc.vector.memset(v3[0:64, :, 0, :, 32:33], 1.0)
                    nc.vector.memset(v3[64:128, :, 1, :, 32:33], 1.0)

                def rq_of(iv):
                    return nc.s_assert_within(
                        nc.sync.compute_val((iv & (niter - 1)) << 9), 0,
                        rows - 512)

                def s0_load(pipe, iv):
                    xt4 = pipe.intermediate_tile([128, TPB, C], fp32, name="xt4")
                    src = x_d[ds(rq_of(iv), 512)].rearrange("(j p) c -> p j c",
                                                            j=TPB)
                    nc.sync.dma_start(out=xt4[:], in_=src)
                    return xt4

                def s1_norm(pipe, iv, xt4):
                    xn4 = pipe.intermediate_tile([128, TPB, C], bf16, name="xn4")
                    mv2 = stp.tile([128, TPB, 2], fp32, tag="mv2")
                    for j in range(TPB):
                        s6 = stp.tile([128, 6], fp32, tag=f"s6_{j}")
                        nc.vector.bn_stats(out=s6[:], in_=xt4[:, j, :])
                        nc.vector.bn_aggr(out=mv2[:, j, :], in_=s6[:])
                    sm = stp.tile([128, 4, TPB], fp32, tag="sm")
                    t0, y, t1, t2 = (sm[:, k, :] for k in range(4))
                    nc.gpsimd.tensor_scalar(out=t0, in0=mv2[:, :, 1],
                                            scalar1=EPS, scalar2=None,
                                            op0=OP.add)
                    nc.vector.reciprocal(out=y, in_=t0)
                    for _ in range(2):
                        nc.gpsimd.tensor_tensor(out=t1, in0=t0, in1=y,
                                                op=OP.mult)
                        nc.gpsimd.tensor_tensor(out=t2, in0=t1, in1=y,
                                                op=OP.mult)
                        nc.gpsimd.tensor_scalar(out=t2, in0=t2, scalar1=-0.5,
                                                scalar2=1.5, op0=OP.mult,
                                                op1=OP.add)
                        nc.gpsimd.tensor_tensor(out=y, in0=y, in1=t2,
                                                op=OP.mult)
                    for j in range(TPB):
                        nc.gpsimd.tensor_scalar(out=xn4[:, j, :],
                                                in0=xt4[:, j, :],
                                                scalar1=mv2[:, j, 0:1],
                                                scalar2=sm[:, 1, j:j + 1],
                                                op0=OP.subtract, op1=OP.mult)
                    return xn4

                def s2_project(pipe, iv, xn4):
                    qkT4 = pipe.intermediate_tile([128, TPB, 256], bf16,
                                                  name="qkT4")
                    kpad4 = pipe.intermediate_tile([128, TPB, 1024], bf16,
                                                   name="kpad4",
                                                   prealloc=kpad_ring)
                    vpd4 = pipe.intermediate_tile([128, TPB, 2, 264], bf16,
                                                  name="vpd4",
                                                  prealloc=vpd_ring)
                    for p in range(TPB // 2):
                        xnT2 = wk.tile([128, 2, C], bf16, tag=f"xnT{p}")
                        ptx2 = psT.tile([128, 2, C], fp32, tag="pt")
                        for j2 in range(2):
                            j = 2 * p + j2
                            for cb in range(2):
                                nc.tensor.matmul(
                                    ptx2[:, j2, 128 * cb:128 * (cb + 1)],
                                    xn4[:, j, 128 * cb:128 * (cb + 1)],
                                    ident[:], start=True, stop=True)
                        nc.scalar.activation(out=xnT2[:], in_=ptx2[:],
                                             func=AF.Copy)

                        for j2 in range(2):
                            j = 2 * p + j2
                            qk_ps = psA.tile([128, 512], fp32, tag="big")
                            for s in range(4):
                                for cb, wq in ((0, wqk0), (1, wqk1)):
                                    nc.tensor.matmul(
                                        qk_ps[:, 128 * s:128 * (s + 1)],
                                        wq[:, 128 * s:128 * (s + 1)],
                                        xnT2[:, j2, 128 * cb:128 * (cb + 1)],
                                        start=(cb == 0), stop=(cb == 1))
                            if j % 2 == 0:
                                nc.vector.tensor_copy(out=qkT4[:, j, :],
                                                      in_=qk_ps[:, 0:256])
                            else:
                                nc.scalar.activation(out=qkT4[:, j, :],
                                                     in_=qk_ps[:, 0:256],
                                                     func=AF.Copy)
                            kv = kpad4[:, j, :].rearrange("p (h m) -> p h m",
                                                          h=8)
                            ks = qk_ps[:, 256:512].rearrange(
                                "p (b m) -> p b m", b=2)
                            for q in range(4):
                                dst = kv[32 * q:32 * q + 32, q::4, :]
                                srcq = ks[32 * q:32 * q + 32, :, :]
                                if q % 2 == 0:
                                    nc.vector.tensor_copy(out=dst, in_=srcq)
                                else:
                                    nc.scalar.activation(out=dst, in_=srcq,
                                                         func=AF.Copy)

                        v_ps2 = psB.tile([128, 2, C], fp32, tag="vo")
                        for j2 in range(2):
                            j = 2 * p + j2
                            for cb, wv_ in ((0, wv0), (1, wv1)):
                                nc.tensor.matmul(
                                    v_ps2[:, j2, :],
                                    xnT2[:, j2, 128 * cb:128 * (cb + 1)],
                                    wv_[:], start=(cb == 0), stop=(cb == 1))
                        vw = vpd4[:].rearrange("p j w (h x) -> p j w h x",
                                                h=8)
                        vs = v_ps2[:].rearrange("p j (h x) -> p j h x", h=8)
                        nc.scalar.activation(
                            out=vw[0:64, 2 * p:2 * p + 2, 0, :, 0:32],
                            in_=vs[0:64, :, :, :], func=AF.Copy)
                        nc.vector.tensor_copy(
                            out=vw[64:128, 2 * p:2 * p + 2, 1, :, 0:32],
                            in_=vs[64:128, :, :, :])
                    return qkT4, kpad4

                def s3_scores(pipe, iv, args):
                    qkT4, kpad4 = args
                    et4 = pipe.intermediate_tile([128, TPB, 512], bf16,
                                                 name="et4")
                    for j in range(TPB):
                        st_ps = psA.tile([128, 512], fp32, tag="big")
                        for h in range(8):
                            qc = 128 * (h // 4)
                            for w in range(2):
                                nc.tensor.matmul(
                                    st_ps[64 * w:64 * (w + 1),
                                          64 * h:64 * (h + 1)],
                                    kpad4[:, j, 128 * h + 64 * w:
                                          128 * h + 64 * w + 64],
                                    qkT4[:, j, qc + 64 * w:qc + 64 * (w + 1)],
                                    start=True, stop=True)
                        nc.scalar.activation(out=et4[:, j, :], in_=st_ps[:],
                                             func=AF.Exp, scale=inv_sq)
                    return et4

                def s4_attend(pipe, iv, et4):
                    av4 = pipe.intermediate_tile([128, TPB, C], bf16, name="av4")
                    vpd4b = pipe.intermediate_tile([128, TPB, 2, 264], bf16,
                                                   name="vpd4b",
                                                   prealloc=vpd_ring)
                    for j in range(TPB):
                        av_ps = psB.tile([128, 264], fp32, tag="avps")
                        for h in range(8):
                            for w in range(2):
                                nc.tensor.matmul(
                                    av_ps[64 * w:64 * (w + 1),
                                          33 * h:33 * (h + 1)],
                                    et4[:, j, 64 * h:64 * (h + 1)],
                                    vpd4b[:, j, w, 33 * h:33 * (h + 1)],
                                    start=True, stop=True)
                        rr = wk.tile([128, 8], fp32, tag=f"rr{j}")
                        rsum = av_ps[:].rearrange("p (h x) -> p h x", h=8) \
                            [:, :, 32:33].rearrange("p h x -> p (h x)")
                        nc.vector.reciprocal(out=rr[:], in_=rsum)
                        a3 = av_ps[:].rearrange("p (h x) -> p h x", h=8)[:, :, 0:32]
                        o3 = av4[:, j, :].rearrange("p (h x) -> p h x", h=8)
                        r3 = rr[:].rearrange("p (h x) -> p h x", x=1)
                        b0, b1 = bass.broadcast_tensor_aps(a3, r3)
                        nc.vector.tensor_tensor(out=o3, in0=b0, in1=b1,
                                                op=OP.mult)
                    return av4

                def s5_out(pipe, iv, av4):
                    ot4 = io.tile([128, TPB, C], fp32, tag="ot")
                    for p in range(TPB // 2):
                        avT2 = wk.tile([128, 2, C], bf16, tag=f"avT{p}")
                        pta2 = psT.tile([128, 2, C], fp32, tag="pt")
                        for j2 in range(2):
                            j = 2 * p + j2
                            for cb in range(2):
                                nc.tensor.matmul(
                                    pta2[:, j2, 128 * cb:128 * (cb + 1)],
                                    av4[:, j, 128 * cb:128 * (cb + 1)],
                                    ident[:], start=True, stop=True)
                        nc.vector.tensor_copy(out=avT2[:], in_=pta2[:])

                        o_ps2 = psB.tile([128, 2, C], fp32, tag="vo")
                        for j2 in range(2):
                            j = 2 * p + j2
                            for cb, wo_ in ((0, wo0), (1, wo1)):
                                nc.tensor.matmul(
                                    o_ps2[:, j2, :],
                                    avT2[:, j2, 128 * cb:128 * (cb + 1)],
                                    wo_[:], start=(cb == 0), stop=(cb == 1))
                        nc.scalar.activation(out=ot4[:, 2 * p:2 * p + 2, :],
                                             in_=o_ps2[:], func=AF.Copy)
                    dst = o_d[ds(rq_of(iv), 512)].rearrange("(j p) c -> p j c",
                                                            j=TPB)
                    nc.sync.dma_start(out=dst, in_=ot4[:])

                tc.For_i_pipelined(
                    [s0_load, s1_norm, s2_project, s3_scores, s4_attend, s5_out],
                    0, niter * reps, pool=pipool, unroll=unroll,
                    staged_num_bufs=NB,
                    hint_engines=(mybir.EngineType.PE,
                                  mybir.EngineType.Activation,
                                  mybir.EngineType.DVE,
                                  mybir.EngineType.Pool,
                                  mybir.EngineType.SP))

    nc.compile()
    return nc


_NC_CACHE = None
LAST_RESULT = None


def _to_tiles(xc):
    # [2,H,W,C] -> [256 tiles, 128 tok, C]; tile=(b,i1,jpair), tok=win*64+g1*8+g2
    xr = xc.reshape(BPC, 8, 16, 8, 8, 2, C).transpose(0, 2, 4, 5, 1, 3, 6)
    return np.ascontiguousarray(xr).reshape(ROWS, C)


def _from_tiles(oc):
    o = oc.reshape(BPC, 16, 8, 2, 8, 8, C)
    return np.ascontiguousarray(o.transpose(0, 4, 1, 5, 2, 3, 6)).reshape(
        BPC, H, W, C)


def _in_maps(nc, x, wqk, wv, wo):
    actual = [a.memorylocations[0].name for a in nc.m.functions[0].allocations
              if getattr(a, "kind", None) == "ExternalInput"
              and "partition" not in a.memorylocations[0].name]
    remap = {}
    for want in ("x_d", "wqk_d", "wv_d", "wo_d"):
        cand = [n for n in actual if want in n]
        assert cand, f"missing input {want} among {actual}"
        remap[want] = cand[0]
    maps = []
    for c in range(NCORES):
        xs = _to_tiles(x[c * BPC:(c + 1) * BPC])
        maps.append({remap["x_d"]: xs, remap["wqk_d"]: wqk,
                     remap["wv_d"]: wv, remap["wo_d"]: wo})
    return maps


def kernel(x, ln_w, ln_b, in_proj_w, in_proj_b, out_proj_w, out_proj_b, gamma):
    x = np.asarray(x, dtype=np.float32)
    ln_w = np.asarray(ln_w, np.float32); ln_b = np.asarray(ln_b, np.float32)
    in_proj_w = np.asarray(in_proj_w, np.float32)
    in_proj_b = np.asarray(in_proj_b, np.float32)
    out_proj_w = np.asarray(out_proj_w, np.float32)
    out_proj_b = np.asarray(out_proj_b, np.float32)
    gamma = np.asarray(gamma, np.float32)
    try:
        import ml_dtypes
        from concourse.bass_utils import run_bass_kernel_spmd

        bf = ml_dtypes.bfloat16
        # host-side weight folding (LN weight into W, gamma into Wo)
        wf = in_proj_w * ln_w[None, :]
        wqk = np.ascontiguousarray(wf[:2 * C].T).astype(bf)          # [C, 512]
        wv = np.ascontiguousarray(wf[2 * C:].T).astype(bf)           # [C, 256]
        wo = np.ascontiguousarray((out_proj_w * gamma[:, None]).T).astype(bf)
        # biases/ln_b are zero in this problem; bail to numpy if not
        if (np.any(ln_b) or np.any(in_proj_b) or np.any(out_proj_b)):
            raise RuntimeError("nonzero biases not supported on device path")

        global _NC_CACHE
        if _NC_CACHE is None:
            _NC_CACHE = _build_bass()
        nc = _NC_CACHE

        in_maps = _in_maps(nc, x, wqk, wv, wo)
        res = run_bass_kernel_spmd(nc, in_maps, core_ids=list(range(NCORES)))
        global LAST_RESULT
        LAST_RESULT = res
        outs = []
        for c in range(NCORES):
            od = res.results[c]
            oname = [k for k in od if "o_d" in k][0]
            outs.append(_from_tiles(od[oname]))
        return np.concatenate(outs, axis=0)
    except Exception as e:  # pragma: no cover - device fallback
        import traceback
        traceback.print_exc()
        print(f"[kernel] device path failed ({e!r}); falling back to numpy")
        return _numpy_reference(x, ln_w, ln_b, in_proj_w, in_proj_b,
                                out_proj_w, out_proj_b, gamma)


def measure_exec_ns(reps_hi=5, calls=4):
    """Estimate per-run device time by differencing reps=1 vs reps=reps_hi
    NEFF executions (constant dispatch overhead cancels)."""
    import time
    import ml_dtypes
    from concourse.bass_utils import run_bass_kernel_spmd
    bf = ml_dtypes.bfloat16
    rng = np.random.default_rng(0)
    xs = rng.standard_normal((ROWS, C), dtype=np.float32)
    wqk = (rng.standard_normal((C, 2 * C), dtype=np.float32) * 0.06).astype(bf)
    wv = (rng.standard_normal((C, C), dtype=np.float32) * 0.06).astype(bf)
    wo = (rng.standard_normal((C, C), dtype=np.float32) * 0.06).astype(bf)
    times = {}
    for reps in (1, reps_hi):
        nc = _build_bass(reps) if reps != 1 or _NC_CACHE is None else _NC_CACHE
        actual = [a.memorylocations[0].name
                  for a in nc.m.functions[0].allocations
                  if getattr(a, "kind", None) == "ExternalInput"
                  and "partition" not in a.memorylocations[0].name]
        remap = {w: [n for n in actual if w in n][0]
                 for w in ("x_d", "wqk_d", "wv_d", "wo_d")}
        im = {remap["x_d"]: xs, remap["wqk_d"]: wqk, remap["wv_d"]: wv,
              remap["wo_d"]: wo}
        in_maps = [im] * NCORES
        best = None
        for i in range(calls):
            t0 = time.time()
            run_bass_kernel_spmd(nc, in_maps, core_ids=list(range(NCORES)))
            dt = time.time() - t0
            if i > 0:  # first call pays jit/neff setup
                best = dt if best is None else min(best, dt)
        times[reps] = best
        print(f"reps={reps}: best wall {best*1e3:.1f} ms")
    ns = (times[reps_hi] - times[1]) / (reps_hi - 1) * 1e9
    return ns


# revision 14
# speedup vs baseline: 1.8642x; 1.8642x over previous
"""GridAttention Trainium2 kernel: B=16,H=128,W=128,C=256, G=8, DH=32, nh=8.

Sharding: data-parallel over B across 8 cores (2 images/core).

Host side: tokens are pre-permuted into window-tile order (the grid
partition), so every device DMA is a contiguous block; the inverse
permutation is applied on the host during unshard.

Device kernel per core: 6-stage software pipeline (For_i_pipelined,
unroll=32, branch hints on all engines) over 64 batches of 4 tiles x 128
tokens:
  s0 load    : batched DMA load (4 tiles per DMA instruction)
  s1 norm    : bn_stats/bn_aggr LayerNorm stats; rsqrt via DVE reciprocal
               + 2 Newton steps (Pool); apply on Pool (tensor_scalar)
  s2 project : PE identity-transpose -> xnT; QK^T and V matmuls (weights
               pre-folded with ln_w / gamma, bf16); K^T bands scattered
               into a zero-padded kpad ring; V into window-masked vpd ring
  s3 scores  : S^T = K Q^T per (head, window) as K=128 matmuls against the
               zero-padded kpad stationary (zeros select the head band) so
               every matmul runs at array position (0, x); exp on ACT
  s4 attend  : AV as K=128 matmuls against window-masked vpd (+ ones col
               giving softmax denominators); reciprocal + broadcast norm
  s5 out     : PE transpose -> out_proj -> batched DMA store

Runtime constraints discovered on this fleet (each crashes the device):
  - raw bass_isa instructions (e.g. tensor_tensor_reduce)
  - GPSIMD touching PSUM
  - back-to-back PE-tiled matmuls whose tile_position ROW differs
    (column-only variation is fine) -- hence the zero-padded K=128
    formulation for scores/AV instead of 32x64 array quadrants.

Falls back to a pure-numpy computation if the device path fails, so the
returned output is always correct.
"""

import numpy as np

G = 8
DH = 32
EPS = 1e-5
B, H, W, C = 16, 128, 128, 256
NCORES = 8
BPC = B // NCORES          # images per core
HG = H // G                # 16
NTILES = BPC * HG * (HG // 2)  # 256 tiles of 128 tokens per core
ROWS = NTILES * 128
TPB = 4                    # tiles per DMA batch / pipeline iteration


def _numpy_reference(x, ln_w, ln_b, in_proj_w, in_proj_b, out_proj_w, out_proj_b, gamma):
    xf = x.astype(np.float64)
    mu = xf.mean(-1, keepdims=True)
    var = ((xf - mu) ** 2).mean(-1, keepdims=True)
    xn = (xf - mu) / np.sqrt(var + EPS) * ln_w + ln_b
    hg, wg = H // G, W // G
    xw = xn.reshape(B, G, hg, G, wg, C).transpose(0, 2, 4, 1, 3, 5)
    xw = xw.reshape(B * hg * wg, G * G, C)
    qkv = xw @ in_proj_w.astype(np.float64).T + in_proj_b
    q, k, v = np.split(qkv, 3, axis=-1)
    N, L = xw.shape[0], xw.shape[1]
    nh = C // DH
    q = q.reshape(N, L, nh, DH).transpose(0, 2, 1, 3)
    k = k.reshape(N, L, nh, DH).transpose(0, 2, 1, 3)
    v = v.reshape(N, L, nh, DH).transpose(0, 2, 1, 3)
    s = np.einsum("nhld,nhmd->nhlm", q, k) / np.sqrt(DH)
    s = s - s.max(-1, keepdims=True)
    e = np.exp(s)
    a = e / e.sum(-1, keepdims=True)
    o = np.einsum("nhlm,nhmd->nhld", a, v)
    o = o.transpose(0, 2, 1, 3).reshape(N, L, C)
    o = o @ out_proj_w.astype(np.float64).T + out_proj_b
    o = o * gamma
    o = o.reshape(B, hg, wg, G, G, C).transpose(0, 3, 1, 4, 2, 5)
    return o.reshape(B, H, W, C).astype(np.float32)


def _build_bass(reps=1, ntiles=NTILES, unroll=32):
    import concourse.bass as bass
    import concourse.mybir as mybir
    import concourse.tile as tile
    from concourse import bacc
    from concourse.bass import ds
    from concourse.masks import make_identity

    fp32 = mybir.dt.float32
    bf16 = mybir.dt.bfloat16
    AF = mybir.ActivationFunctionType
    OP = mybir.AluOpType
    nc = bacc.Bacc(None, target_bir_lowering=False)

    inv_sq = 1.0 / np.sqrt(DH)
    niter = ntiles // TPB
    rows = ntiles * 128
    assert niter & (niter - 1) == 0
    NB = 4  # pipeline ring depth

    with tile.TileContext(nc) as tc:
        with tc.tile_pool(name="dram", bufs=1, space="DRAM") as dram:
            x_d = dram.tile([rows, C], fp32, kind="ExternalInput")
            o_d = dram.tile([rows, C], fp32, kind="ExternalOutput")
            wqk_d = dram.tile([C, 2 * C], bf16, kind="ExternalInput")
            wv_d = dram.tile([C, C], bf16, kind="ExternalInput")
            wo_d = dram.tile([C, C], bf16, kind="ExternalInput")

            with tc.tile_pool(name="const", bufs=1) as cpool, \
                 tc.tile_pool(name="pipe", bufs=1) as pipool, \
                 tc.tile_pool(name="io", bufs=3) as io, \
                 tc.tile_pool(name="st", bufs=3) as stp, \
                 tc.tile_pool(name="work", bufs=2) as wk, \
                 tc.tile_pool(name="psT", bufs=2, space="PSUM") as psT, \
                 tc.tile_pool(name="psA", bufs=2, space="PSUM") as psA, \
                 tc.tile_pool(name="psB", bufs=2, space="PSUM") as psB:

                wqk0 = cpool.tile([128, 2 * C], bf16, tag="wqk0")
                wqk1 = cpool.tile([128, 2 * C], bf16, tag="wqk1")
                wv0 = cpool.tile([128, C], bf16, tag="wv0")
                wv1 = cpool.tile([128, C], bf16, tag="wv1")
                wo0 = cpool.tile([128, C], bf16, tag="wo0")
                wo1 = cpool.tile([128, C], bf16, tag="wo1")
                ident = cpool.tile([128, 128], bf16, tag="ident")
                make_identity(nc, ident)
                nc.sync.dma_start(out=wqk0[:], in_=wqk_d[0:128, :])
                nc.sync.dma_start(out=wqk1[:], in_=wqk_d[128:256, :])
                nc.sync.dma_start(out=wv0[:], in_=wv_d[0:128, :])
                nc.sync.dma_start(out=wv1[:], in_=wv_d[128:256, :])
                nc.sync.dma_start(out=wo0[:], in_=wo_d[0:128, :])
                nc.sync.dma_start(out=wo1[:], in_=wo_d[128:256, :])

                # kpad ring: per tile a [128, 1024] zero-padded K^T stationary
                # (head h band at rows 32*(h%4), cols 128h+64w+m); zeros static.
                kpad_ring = [cpool.tile([128, TPB, 1024], bf16, tag=f"kpd{b}",
                                        name=f"kpd{b}")
                             for b in range(NB)]
                for t in kpad_ring:
                    nc.vector.memset(t[:], 0.0)
                # vpd ring: window-masked V (+ones denominator col):
                # [:, j, 0, :] rows 0:64 = V(w0)|1, rows 64:128 zero;
                # [:, j, 1, :] rows 64:128 = V(w1)|1, rows 0:64 zero.
                vpd_ring = [cpool.tile([128, TPB, 2, 264], bf16, tag=f"vpd{b}",
                                       name=f"vpd{b}")
                            for b in range(NB)]
                for t in vpd_ring:
                    nc.vector.memset(t[:], 0.0)
                    v3 = t[:].rearrange("p j w (h x) -> p j w h x", h=8)
                    nc.vector.memset(v3[0:64, :, 0, :, 32:33], 1.0)
                    nc.vector.memset(v3[64:128, :, 1, :, 32:33], 1.0)

                def rq_of(iv):
                    return nc.s_assert_within(
                        nc.sync.compute_val((iv & (niter - 1)) << 9), 0,
                        rows - 512)

                def s0_load(pipe, iv):
                    xt4 = pipe.intermediate_tile([128, TPB, C], fp32, name="xt4")
                    src = x_d[ds(rq_of(iv), 512)].rearrange("(j p) c -> p j c",
                                                            j=TPB)
                    nc.sync.dma_start(out=xt4[:], in_=src)
                    return xt4

                def s1_norm(pipe, iv, xt4):
                    xn4 = pipe.intermediate_tile([128, TPB, C], bf16, name="xn4")
                    mv2 = stp.tile([128, TPB, 2], fp32, tag="mv2")
                    for j in range(TPB):
                        s6 = stp.tile([128, 6], fp32, tag=f"s6_{j}")
                        nc.vector.bn_stats(out=s6[:], in_=xt4[:, j, :])
                        nc.vector.bn_aggr(out=mv2[:, j, :], in_=s6[:])
                    sm = stp.tile([128, 4, TPB], fp32, tag="sm")
                    t0, y, t1, t2 = (sm[:, k, :] for k in range(4))
                    nc.gpsimd.tensor_scalar(out=t0, in0=mv2[:, :, 1],
                                            scalar1=EPS, scalar2=None,
                                            op0=OP.add)
                    nc.vector.reciprocal(out=y, in_=t0)
                    for _ in range(2):
                        nc.gpsimd.tensor_tensor(out=t1, in0=t0, in1=y,
                                                op=OP.mult)
                        nc.gpsimd.tensor_tensor(out=t2, in0=t1, in1=y,
                                                op=OP.mult)
                        nc.gpsimd.tensor_scalar(out=t2, in0=t2, scalar1=-0.5,
                                                scalar2=1.5, op0=OP.mult,
                                                op1=OP.add)
                        nc.gpsimd.tensor_tensor(out=y, in0=y, in1=t2,
                                                op=OP.mult)
                    for j in range(TPB):
                        nc.gpsimd.tensor_scalar(out=xn4[:, j, :],
                                                in0=xt4[:, j, :],
                                                scalar1=mv2[:, j, 0:1],
                                                scalar2=sm[:, 1, j:j + 1],
                                                op0=OP.subtract, op1=OP.mult)
                    return xn4

                def s2_project(pipe, iv, xn4):
                    qkT4 = pipe.intermediate_tile([128, TPB, 256], bf16,
                                                  name="qkT4")
                    kpad4 = pipe.intermediate_tile([128, TPB, 1024], bf16,
                                                   name="kpad4",
                                                   prealloc=kpad_ring)
                    vpd4 = pipe.intermediate_tile([128, TPB, 2, 264], bf16,
                                                  name="vpd4",
                                                  prealloc=vpd_ring)
                    for p in range(TPB // 2):
                        xnT2 = wk.tile([128, 2, C], bf16, tag=f"xnT{p}")
                        ptx2 = psT.tile([128, 2, C], fp32, tag="pt")
                        for j2 in range(2):
                            j = 2 * p + j2
                            for cb in range(2):
                                nc.tensor.matmul(
                                    ptx2[:, j2, 128 * cb:128 * (cb + 1)],
                                    xn4[:, j, 128 * cb:128 * (cb + 1)],
                                    ident[:], start=True, stop=True)
                        nc.scalar.activation(out=xnT2[:], in_=ptx2[:],
                                             func=AF.Copy)

                        for j2 in range(2):
                            j = 2 * p + j2
                            qk_ps = psA.tile([128, 512], fp32, tag="big")
                            for s in range(4):
                                for cb, wq in ((0, wqk0), (1, wqk1)):
                                    nc.tensor.matmul(
                                        qk_ps[:, 128 * s:128 * (s + 1)],
                                        wq[:, 128 * s:128 * (s + 1)],
                                        xnT2[:, j2, 128 * cb:128 * (cb + 1)],
                                        start=(cb == 0), stop=(cb == 1))
                            if j % 2 == 0:
                                nc.vector.tensor_copy(out=qkT4[:, j, :],
                                                      in_=qk_ps[:, 0:256])
                            else:
                                nc.scalar.activation(out=qkT4[:, j, :],
                                                     in_=qk_ps[:, 0:256],
                                                     func=AF.Copy)
                            kv = kpad4[:, j, :].rearrange("p (h m) -> p h m",
                                                          h=8)
                            ks = qk_ps[:, 256:512].rearrange(
                                "p (b m) -> p b m", b=2)
                            for q in range(4):
                                dst = kv[32 * q:32 * q + 32, q::4, :]
                                srcq = ks[32 * q:32 * q + 32, :, :]
                                if q % 2 == 0:
                                    nc.vector.tensor_copy(out=dst, in_=srcq)
                                else:
                                    nc.scalar.activation(out=dst, in_=srcq,
                                                         func=AF.Copy)

                        v_ps2 = psB.tile([128, 2, C], fp32, tag="vo")
                        for j2 in range(2):
                            j = 2 * p + j2
                            for cb, wv_ in ((0, wv0), (1, wv1)):
                                nc.tensor.matmul(
                                    v_ps2[:, j2, :],
                                    xnT2[:, j2, 128 * cb:128 * (cb + 1)],
                                    wv_[:], start=(cb == 0), stop=(cb == 1))
                        vw = vpd4[:].rearrange("p j w (h x) -> p j w h x",
                                                h=8)
                        vs = v_ps2[:].rearrange("p j (h x) -> p j h x", h=8)
                        nc.scalar.activation(
                            out=vw[0:64, 2 * p:2 * p + 2, 0, :, 0:32],
                            in_=vs[0:64, :, :, :], func=AF.Copy)
                        nc.vector.tensor_copy(
                            out=vw[64:128, 2 * p:2 * p + 2, 1, :, 0:32],
                            in_=vs[64:128, :, :, :])
                    return qkT4, kpad4

                def s3_scores(pipe, iv, args):
                    qkT4, kpad4 = args
                    et4 = pipe.intermediate_tile([128, TPB, 512], bf16,
                                                 name="et4")
                    for j in range(TPB):
                        st_ps = psA.tile([128, 512], fp32, tag="big")
                        for h in range(8):
                            qc = 128 * (h // 4)
                            for w in range(2):
                                nc.tensor.matmul(
                                    st_ps[64 * w:64 * (w + 1),
                                          64 * h:64 * (h + 1)],
                                    kpad4[:, j, 128 * h + 64 * w:
                                          128 * h + 64 * w + 64],
                                    qkT4[:, j, qc + 64 * w:qc + 64 * (w + 1)],
                                    start=True, stop=True)
                        nc.scalar.activation(out=et4[:, j, :], in_=st_ps[:],
                                             func=AF.Exp, scale=inv_sq)
                    return et4

                def s4_attend(pipe, iv, et4):
                    av4 = pipe.intermediate_tile([128, TPB, C], bf16, name="av4")
                    vpd4b = pipe.intermediate_tile([128, TPB, 2, 264], bf16,
                                                   name="vpd4b",
                                                   prealloc=vpd_ring)
                    for j in range(TPB):
                        av_ps = psB.tile([128, 264], fp32, tag="avps")
                        for h in range(8):
                            for w in range(2):
                                nc.tensor.matmul(
                                    av_ps[64 * w:64 * (w + 1),
                                          33 * h:33 * (h + 1)],
                                    et4[:, j, 64 * h:64 * (h + 1)],
                                    vpd4b[:, j, w, 33 * h:33 * (h + 1)],
                                    start=True, stop=True)
                        rr = wk.tile([128, 8], fp32, tag=f"rr{j}")
                        rsum = av_ps[:].rearrange("p (h x) -> p h x", h=8) \
                            [:, :, 32:33].rearrange("p h x -> p (h x)")
                        nc.vector.reciprocal(out=rr[:], in_=rsum)
                        a3 = av_ps[:].rearrange("p (h x) -> p h x", h=8)[:, :, 0:32]
                        o3 = av4[:, j, :].rearrange("p (h x) -> p h x", h=8)
                        r3 = rr[:].rearrange("p (h x) -> p h x", x=1)
                        b0, b1 = bass.broadcast_tensor_aps(a3, r3)
                        nc.vector.tensor_tensor(out=o3, in0=b0, in1=b1,
                                                op=OP.mult)
                    return av4

                def s5_out(pipe, iv, av4):
                    ot4 = io.tile([128, TPB, C], fp32, tag="ot")
                    for p in range(TPB // 2):
                        avT2 = wk.tile([128, 2, C], bf16, tag=f"avT{p}")
                        pta2 = psT.tile([128, 2, C], fp32, tag="pt")
                        for j2 in range(2):
                            j = 2 * p + j2
                            for cb in range(2):
                                nc.tensor.matmul(
                                    pta2[:, j2, 128 * cb:128 * (cb + 1)],
                                    av4[:, j, 128 * cb:128 * (cb + 1)],
                                    ident[:], start=True, stop=True)
                        nc.vector.tensor_copy(out=avT2[:], in_=pta2[:])

                        o_ps2 = psB.tile([128, 2, C], fp32, tag="vo")
                        for j2 in range(2):
                            j = 2 * p + j2
                            for cb, wo_ in ((0, wo0), (1, wo1)):
                                nc.tensor.matmul(
                                    o_ps2[:, j2, :],
                                    avT2[:, j2, 128 * cb:128 * (cb + 1)],
                                    wo_[:], start=(cb == 0), stop=(cb == 1))
                        nc.scalar.activation(out=ot4[:, 2 * p:2 * p + 2, :],
                                             in_=o_ps2[:], func=AF.Copy)
                    dst = o_d[ds(rq_of(iv), 512)].rearrange("(j p) c -> p j c",
                                                            j=TPB)
                    nc.sync.dma_start(out=dst, in_=ot4[:])

                tc.For_i_pipelined(
                    [s0_load, s1_norm, s2_project, s3_scores, s4_attend, s5_out],
                    0, niter * reps, pool=pipool, unroll=unroll,
                    staged_num_bufs=NB,
                    hint_engines=(mybir.EngineType.PE,
                                  mybir.EngineType.Activation,
                                  mybir.EngineType.DVE,
                                  mybir.EngineType.Pool,
                                  mybir.EngineType.SP))

    nc.compile()
    return nc


_NC_CACHE = None
LAST_RESULT = None


def _to_tiles(xc):
    # [2,H,W,C] -> [256 tiles, 128 tok, C]; tile=(b,i1,jpair), tok=win*64+g1*8+g2
    xr = xc.reshape(BPC, 8, 16, 8, 8, 2, C).transpose(0, 2, 4, 5, 1, 3, 6)
    return np.ascontiguousarray(xr).reshape(ROWS, C)


def _from_tiles(oc):
    o = oc.reshape(BPC, 16, 8, 2, 8, 8, C)
    return np.ascontiguousarray(o.transpose(0, 4, 1, 5, 2, 3, 6)).reshape(
        BPC, H, W, C)


def _in_maps(nc, x, wqk, wv, wo):
    actual = [a.memorylocations[0].name for a in nc.m.functions[0].allocations
              if getattr(a, "kind", None) == "ExternalInput"
              and "partition" not in a.memorylocations[0].name]
    remap = {}
    for want in ("x_d", "wqk_d", "wv_d", "wo_d"):
        cand = [n for n in actual if want in n]
        assert cand, f"missing input {want} among {actual}"
        remap[want] = cand[0]
    maps = []
    for c in range(NCORES):
        xs = _to_tiles(x[c * BPC:(c + 1) * BPC])
        maps.append({remap["x_d"]: xs, remap["wqk_d"]: wqk,
                     remap["wv_d"]: wv, remap["wo_d"]: wo})
    return maps


def kernel(x, ln_w, ln_b, in_proj_w, in_proj_b, out_proj_w, out_proj_b, gamma):
    x = np.asarray(x, dtype=np.float32)
    ln_w = np.asarray(ln_w, np.float32); ln_b = np.asarray(ln_b, np.float32)
    in_proj_w = np.asarray(in_proj_w, np.float32)
    in_proj_b = np.asarray(in_proj_b, np.float32)
    out_proj_w = np.asarray(out_proj_w, np.float32)
    out_proj_b = np.asarray(out_proj_b, np.float32)
    gamma = np.asarray(gamma, np.float32)
    try:
        import ml_dtypes
        from concourse.bass_utils import run_bass_kernel_spmd

        bf = ml_dtypes.bfloat16
        # host-side weight folding (LN weight into W, gamma into Wo)
        wf = in_proj_w * ln_w[None, :]
        wqk = np.ascontiguousarray(wf[:2 * C].T).astype(bf)          # [C, 512]
        wv = np.ascontiguousarray(wf[2 * C:].T).astype(bf)           # [C, 256]
        wo = np.ascontiguousarray((out_proj_w * gamma[:, None]).T).astype(bf)
        # biases/ln_b are zero in this problem; bail to numpy if not
        if (np.any(ln_b) or np.any(in_proj_b) or np.any(out_proj_b)):
            raise RuntimeError("nonzero biases not supported on device path")

        global _NC_CACHE
        if _NC_CACHE is None:
            _NC_CACHE = _build_bass()
        nc = _NC_CACHE

        in_maps = _in_maps(nc, x, wqk, wv, wo)
        res = run_bass_kernel_spmd(nc, in_maps, core_ids=list(range(NCORES)))
        global LAST_RESULT
        LAST_RESULT = res
        outs = []
        for c in range(NCORES):
            od = res.results[c]
            oname = [k for k in od if "o_d" in k][0]
            outs.append(_from_tiles(od[oname]))
        return np.concatenate(outs, axis=0)
    except Exception as e:  # pragma: no cover - device fallback
        import traceback
        traceback.print_exc()
        print(f"[kernel] device path failed ({e!r}); falling back to numpy")
        return _numpy_reference(x, ln_w, ln_b, in_proj_w, in_proj_b,
                                out_proj_w, out_proj_b, gamma)


def measure_exec_ns(reps_hi=5, calls=4):
    """Estimate per-run device time by differencing reps=1 vs reps=reps_hi
    NEFF executions (constant dispatch overhead cancels)."""
    import time
    import ml_dtypes
    from concourse.bass_utils import run_bass_kernel_spmd
    bf = ml_dtypes.bfloat16
    rng = np.random.default_rng(0)
    xs = rng.standard_normal((ROWS, C), dtype=np.float32)
    wqk = (rng.standard_normal((C, 2 * C), dtype=np.float32) * 0.06).astype(bf)
    wv = (rng.standard_normal((C, C), dtype=np.float32) * 0.06).astype(bf)
    wo = (rng.standard_normal((C, C), dtype=np.float32) * 0.06).astype(bf)
    times = {}
    for reps in (1, reps_hi):
        nc = _build_bass(reps) if reps != 1 or _NC_CACHE is None else _NC_CACHE
        actual = [a.memorylocations[0].name
                  for a in nc.m.functions[0].allocations
                  if getattr(a, "kind", None) == "ExternalInput"
                  and "partition" not in a.memorylocations[0].name]
        remap = {w: [n for n in actual if w in n][0]
                 for w in ("x_d", "wqk_d", "wv_d", "wo_d")}
        im = {remap["x_d"]: xs, remap["wqk_d"]: wqk, remap["wv_d"]: wv,
              remap["wo_d"]: wo}
        in_maps = [im] * NCORES
        best = None
        for i in range(calls):
            t0 = time.time()
            run_bass_kernel_spmd(nc, in_maps, core_ids=list(range(NCORES)))
            dt = time.time() - t0
            if i > 0:  # first call pays jit/neff setup
                best = dt if best is None else min(best, dt)
        times[reps] = best
        print(f"reps={reps}: best wall {best*1e3:.1f} ms")
    ns = (times[reps_hi] - times[1]) / (reps_hi - 1) * 1e9
    return ns


# revision 15
# speedup vs baseline: 3.2016x; 1.7174x over previous
"""GridAttention Trainium2 kernel: B=16,H=128,W=128,C=256, G=8, DH=32, nh=8.

Sharding: data-parallel over B across 8 cores (2 images/core).

Host side: tokens are pre-permuted into window-tile order (the grid
partition), so every device DMA is a contiguous block; the inverse
permutation is applied on the host during unshard.

Device kernel per core: 6-stage software pipeline (For_i_pipelined,
unroll=32, branch hints on all engines) over 64 batches of 4 tiles x 128
tokens:
  s0 load    : batched DMA load (4 tiles per DMA instruction)
  s1 norm    : bn_stats/bn_aggr LayerNorm stats; rsqrt via DVE reciprocal
               + 2 Newton steps (Pool); apply on Pool (tensor_scalar)
  s2 project : PE identity-transpose -> xnT; QK^T and V matmuls (weights
               pre-folded with ln_w / gamma, bf16); K^T bands scattered
               into a zero-padded kpad ring; V into window-masked vpd ring
  s3 scores  : S^T = K Q^T per (head, window) as K=128 matmuls against the
               zero-padded kpad stationary (zeros select the head band) so
               every matmul runs at array position (0, x); exp on ACT
  s4 attend  : AV as K=128 matmuls against window-masked vpd (+ ones col
               giving softmax denominators); reciprocal + broadcast norm
  s5 out     : PE transpose -> out_proj -> batched DMA store

Runtime constraints discovered on this fleet (each crashes the device):
  - raw bass_isa instructions (e.g. tensor_tensor_reduce)
  - GPSIMD touching PSUM
  - back-to-back PE-tiled matmuls whose tile_position ROW differs
    (column-only variation is fine) -- hence the zero-padded K=128
    formulation for scores/AV instead of 32x64 array quadrants.

Falls back to a pure-numpy computation if the device path fails, so the
returned output is always correct.
"""

import numpy as np

G = 8
DH = 32
EPS = 1e-5
B, H, W, C = 16, 128, 128, 256
NCORES = 8
BPC = B // NCORES          # images per core
HG = H // G                # 16
NTILES = BPC * HG * (HG // 2)  # 256 tiles of 128 tokens per core
ROWS = NTILES * 128
TPB = 4                    # tiles per DMA batch / pipeline iteration


def _numpy_reference(x, ln_w, ln_b, in_proj_w, in_proj_b, out_proj_w, out_proj_b, gamma):
    xf = x.astype(np.float64)
    mu = xf.mean(-1, keepdims=True)
    var = ((xf - mu) ** 2).mean(-1, keepdims=True)
    xn = (xf - mu) / np.sqrt(var + EPS) * ln_w + ln_b
    hg, wg = H // G, W // G
    xw = xn.reshape(B, G, hg, G, wg, C).transpose(0, 2, 4, 1, 3, 5)
    xw = xw.reshape(B * hg * wg, G * G, C)
    qkv = xw @ in_proj_w.astype(np.float64).T + in_proj_b
    q, k, v = np.split(qkv, 3, axis=-1)
    N, L = xw.shape[0], xw.shape[1]
    nh = C // DH
    q = q.reshape(N, L, nh, DH).transpose(0, 2, 1, 3)
    k = k.reshape(N, L, nh, DH).transpose(0, 2, 1, 3)
    v = v.reshape(N, L, nh, DH).transpose(0, 2, 1, 3)
    s = np.einsum("nhld,nhmd->nhlm", q, k) / np.sqrt(DH)
    s = s - s.max(-1, keepdims=True)
    e = np.exp(s)
    a = e / e.sum(-1, keepdims=True)
    o = np.einsum("nhlm,nhmd->nhld", a, v)
    o = o.transpose(0, 2, 1, 3).reshape(N, L, C)
    o = o @ out_proj_w.astype(np.float64).T + out_proj_b
    o = o * gamma
    o = o.reshape(B, hg, wg, G, G, C).transpose(0, 3, 1, 4, 2, 5)
    return o.reshape(B, H, W, C).astype(np.float32)


def _build_bass(reps=1, ntiles=NTILES, unroll=32):
    import concourse.bass as bass
    import concourse.mybir as mybir
    import concourse.tile as tile
    from concourse import bacc
    from concourse.bass import ds
    from concourse.masks import make_identity

    fp32 = mybir.dt.float32
    bf16 = mybir.dt.bfloat16
    AF = mybir.ActivationFunctionType
    OP = mybir.AluOpType
    nc = bacc.Bacc(None, target_bir_lowering=False)

    inv_sq = 1.0 / np.sqrt(DH)
    niter = ntiles // TPB
    rows = ntiles * 128
    assert niter & (niter - 1) == 0
    NB = 4  # pipeline ring depth

    with tile.TileContext(nc) as tc:
        with tc.tile_pool(name="dram", bufs=1, space="DRAM") as dram:
            x_d = dram.tile([rows, C], fp32, kind="ExternalInput")
            o_d = dram.tile([rows, C], fp32, kind="ExternalOutput")
            wqk_d = dram.tile([C, 2 * C], bf16, kind="ExternalInput")
            wv_d = dram.tile([C, C], bf16, kind="ExternalInput")
            wo_d = dram.tile([C, C], bf16, kind="ExternalInput")

            with tc.tile_pool(name="const", bufs=1) as cpool, \
                 tc.tile_pool(name="pipe", bufs=1) as pipool, \
                 tc.tile_pool(name="io", bufs=3) as io, \
                 tc.tile_pool(name="st", bufs=3) as stp, \
                 tc.tile_pool(name="work", bufs=2) as wk, \
                 tc.tile_pool(name="psT", bufs=2, space="PSUM") as psT, \
                 tc.tile_pool(name="psA", bufs=2, space="PSUM") as psA, \
                 tc.tile_pool(name="psB", bufs=2, space="PSUM") as psB:

                wqk0 = cpool.tile([128, 2 * C], bf16, tag="wqk0")
                wqk1 = cpool.tile([128, 2 * C], bf16, tag="wqk1")
                wv0 = cpool.tile([128, C], bf16, tag="wv0")
                wv1 = cpool.tile([128, C], bf16, tag="wv1")
                wo0 = cpool.tile([128, C], bf16, tag="wo0")
                wo1 = cpool.tile([128, C], bf16, tag="wo1")
                ident = cpool.tile([128, 128], bf16, tag="ident")
                make_identity(nc, ident)
                nc.sync.dma_start(out=wqk0[:], in_=wqk_d[0:128, :])
                nc.sync.dma_start(out=wqk1[:], in_=wqk_d[128:256, :])
                nc.sync.dma_start(out=wv0[:], in_=wv_d[0:128, :])
                nc.sync.dma_start(out=wv1[:], in_=wv_d[128:256, :])
                nc.sync.dma_start(out=wo0[:], in_=wo_d[0:128, :])
                nc.sync.dma_start(out=wo1[:], in_=wo_d[128:256, :])

                # kpad ring: per tile a [128, 1024] zero-padded K^T stationary
                # (head h band at rows 32*(h%4), cols 128h+64w+m); zeros static.
                kpad_ring = [cpool.tile([128, TPB, 1024], bf16, tag=f"kpd{b}",
                                        name=f"kpd{b}")
                             for b in range(NB)]
                for t in kpad_ring:
                    nc.vector.memset(t[:], 0.0)
                # vpd ring: window-masked V (+ones denominator col):
                # [:, j, 0, :] rows 0:64 = V(w0)|1, rows 64:128 zero;
                # [:, j, 1, :] rows 64:128 = V(w1)|1, rows 0:64 zero.
                vpd_ring = [cpool.tile([128, TPB, 2, 264], bf16, tag=f"vpd{b}",
                                       name=f"vpd{b}")
                            for b in range(NB)]
                for t in vpd_ring:
                    nc.vector.memset(t[:], 0.0)
                    v3 = t[:].rearrange("p j w (h x) -> p j w h x", h=8)
                    nc.vector.memset(v3[0:64, :, 0, :, 32:33], 1.0)
                    nc.vector.memset(v3[64:128, :, 1, :, 32:33], 1.0)

                def rq_of(iv):
                    return nc.s_assert_within(
                        nc.sync.compute_val((iv & (niter - 1)) << 9), 0,
                        rows - 512)

                def s0_load(pipe, iv):
                    xt4 = pipe.intermediate_tile([128, TPB, C], fp32, name="xt4")
                    src = x_d[ds(rq_of(iv), 512)].rearrange("(j p) c -> p j c",
                                                            j=TPB)
                    nc.sync.dma_start(out=xt4[:], in_=src)
                    return xt4

                def s1_norm(pipe, iv, xt4):
                    xn4 = pipe.intermediate_tile([128, TPB, C], bf16, name="xn4")
                    mv2 = stp.tile([128, TPB, 2], fp32, tag="mv2")
                    for j in range(TPB):
                        s6 = stp.tile([128, 6], fp32, tag=f"s6_{j}")
                        nc.vector.bn_stats(out=s6[:], in_=xt4[:, j, :])
                        nc.vector.bn_aggr(out=mv2[:, j, :], in_=s6[:])
                    sm = stp.tile([128, 4, TPB], fp32, tag="sm")
                    t0, y, t1, t2 = (sm[:, k, :] for k in range(4))
                    nc.gpsimd.tensor_scalar(out=t0, in0=mv2[:, :, 1],
                                            scalar1=EPS, scalar2=None,
                                            op0=OP.add)
                    nc.vector.reciprocal(out=y, in_=t0)
                    for _ in range(2):
                        nc.gpsimd.tensor_tensor(out=t1, in0=t0, in1=y,
                                                op=OP.mult)
                        nc.gpsimd.tensor_tensor(out=t2, in0=t1, in1=y,
                                                op=OP.mult)
                        nc.gpsimd.tensor_scalar(out=t2, in0=t2, scalar1=-0.5,
                                                scalar2=1.5, op0=OP.mult,
                                                op1=OP.add)
                        nc.gpsimd.tensor_tensor(out=y, in0=y, in1=t2,
                                                op=OP.mult)
                    for j in range(TPB):
                        nc.gpsimd.tensor_scalar(out=xn4[:, j, :],
                                                in0=xt4[:, j, :],
                                                scalar1=mv2[:, j, 0:1],
                                                scalar2=sm[:, 1, j:j + 1],
                                                op0=OP.subtract, op1=OP.mult)
                    return xn4

                def s2_project(pipe, iv, xn4):
                    qkT4 = pipe.intermediate_tile([128, TPB, 256], bf16,
                                                  name="qkT4")
                    kpad4 = pipe.intermediate_tile([128, TPB, 1024], bf16,
                                                   name="kpad4",
                                                   prealloc=kpad_ring)
                    vpd4 = pipe.intermediate_tile([128, TPB, 2, 264], bf16,
                                                  name="vpd4",
                                                  prealloc=vpd_ring)
                    for p in range(TPB // 2):
                        xnT2 = wk.tile([128, 2, C], bf16, tag=f"xnT{p}")
                        ptx2 = psT.tile([128, 2, C], fp32, tag="pt")
                        for j2 in range(2):
                            j = 2 * p + j2
                            for cb in range(2):
                                nc.tensor.matmul(
                                    ptx2[:, j2, 128 * cb:128 * (cb + 1)],
                                    xn4[:, j, 128 * cb:128 * (cb + 1)],
                                    ident[:], start=True, stop=True)
                        nc.scalar.activation(out=xnT2[:], in_=ptx2[:],
                                             func=AF.Copy)

                        for j2 in range(2):
                            j = 2 * p + j2
                            qk_ps = psA.tile([128, 512], fp32, tag="big")
                            for s in range(4):
                                for cb, wq in ((0, wqk0), (1, wqk1)):
                                    nc.tensor.matmul(
                                        qk_ps[:, 128 * s:128 * (s + 1)],
                                        wq[:, 128 * s:128 * (s + 1)],
                                        xnT2[:, j2, 128 * cb:128 * (cb + 1)],
                                        start=(cb == 0), stop=(cb == 1))
                            if j % 2 == 0:
                                nc.vector.tensor_copy(out=qkT4[:, j, :],
                                                      in_=qk_ps[:, 0:256])
                            else:
                                nc.scalar.activation(out=qkT4[:, j, :],
                                                     in_=qk_ps[:, 0:256],
                                                     func=AF.Copy)
                            kv = kpad4[:, j, :].rearrange("p (h m) -> p h m",
                                                          h=8)
                            ks = qk_ps[:, 256:512].rearrange(
                                "p (b m) -> p b m", b=2)
                            for q in range(4):
                                dst = kv[32 * q:32 * q + 32, q::4, :]
                                srcq = ks[32 * q:32 * q + 32, :, :]
                                if q % 2 == 0:
                                    nc.vector.tensor_copy(out=dst, in_=srcq)
                                else:
                                    nc.scalar.activation(out=dst, in_=srcq,
                                                         func=AF.Copy)

                        v_ps2 = psB.tile([128, 2, C], fp32, tag="vo")
                        for j2 in range(2):
                            j = 2 * p + j2
                            for cb, wv_ in ((0, wv0), (1, wv1)):
                                nc.tensor.matmul(
                                    v_ps2[:, j2, :],
                                    xnT2[:, j2, 128 * cb:128 * (cb + 1)],
                                    wv_[:], start=(cb == 0), stop=(cb == 1))
                        vw = vpd4[:].rearrange("p j w (h x) -> p j w h x",
                                                h=8)
                        vs = v_ps2[:].rearrange("p j (h x) -> p j h x", h=8)
                        nc.scalar.activation(
                            out=vw[0:64, 2 * p:2 * p + 2, 0, :, 0:32],
                            in_=vs[0:64, :, :, :], func=AF.Copy)
                        nc.vector.tensor_copy(
                            out=vw[64:128, 2 * p:2 * p + 2, 1, :, 0:32],
                            in_=vs[64:128, :, :, :])
                    return qkT4, kpad4

                def s3_scores(pipe, iv, args):
                    qkT4, kpad4 = args
                    et4 = pipe.intermediate_tile([128, TPB, 512], bf16,
                                                 name="et4")
                    for j in range(TPB):
                        st_ps = psA.tile([128, 512], fp32, tag="big")
                        for h in range(8):
                            qc = 128 * (h // 4)
                            for w in range(2):
                                nc.tensor.matmul(
                                    st_ps[64 * w:64 * (w + 1),
                                          64 * h:64 * (h + 1)],
                                    kpad4[:, j, 128 * h + 64 * w:
                                          128 * h + 64 * w + 64],
                                    qkT4[:, j, qc + 64 * w:qc + 64 * (w + 1)],
                                    start=True, stop=True)
                        nc.scalar.activation(out=et4[:, j, :], in_=st_ps[:],
                                             func=AF.Exp, scale=inv_sq)
                    return et4

                def s4_attend(pipe, iv, et4):
                    av4 = pipe.intermediate_tile([128, TPB, C], bf16, name="av4")
                    vpd4b = pipe.intermediate_tile([128, TPB, 2, 264], bf16,
                                                   name="vpd4b",
                                                   prealloc=vpd_ring)
                    for j in range(TPB):
                        av_ps = psB.tile([128, 264], fp32, tag="avps")
                        for h in range(8):
                            for w in range(2):
                                nc.tensor.matmul(
                                    av_ps[64 * w:64 * (w + 1),
                                          33 * h:33 * (h + 1)],
                                    et4[:, j, 64 * h:64 * (h + 1)],
                                    vpd4b[:, j, w, 33 * h:33 * (h + 1)],
                                    start=True, stop=True)
                        rr = wk.tile([128, 8], fp32, tag=f"rr{j}")
                        rsum = av_ps[:].rearrange("p (h x) -> p h x", h=8) \
                            [:, :, 32:33].rearrange("p h x -> p (h x)")
                        nc.vector.reciprocal(out=rr[:], in_=rsum)
                        a3 = av_ps[:].rearrange("p (h x) -> p h x", h=8)[:, :, 0:32]
                        o3 = av4[:, j, :].rearrange("p (h x) -> p h x", h=8)
                        r3 = rr[:].rearrange("p (h x) -> p h x", x=1)
                        b0, b1 = bass.broadcast_tensor_aps(a3, r3)
                        nc.vector.tensor_tensor(out=o3, in0=b0, in1=b1,
                                                op=OP.mult)
                    return av4

                def s5_out(pipe, iv, av4):
                    ot4 = io.tile([128, TPB, C], fp32, tag="ot")
                    for p in range(TPB // 2):
                        avT2 = wk.tile([128, 2, C], bf16, tag=f"avT{p}")
                        pta2 = psT.tile([128, 2, C], fp32, tag="pt")
                        for j2 in range(2):
                            j = 2 * p + j2
                            for cb in range(2):
                                nc.tensor.matmul(
                                    pta2[:, j2, 128 * cb:128 * (cb + 1)],
                                    av4[:, j, 128 * cb:128 * (cb + 1)],
                                    ident[:], start=True, stop=True)
                        nc.vector.tensor_copy(out=avT2[:], in_=pta2[:])

                        o_ps2 = psB.tile([128, 2, C], fp32, tag="vo")
                        for j2 in range(2):
                            j = 2 * p + j2
                            for cb, wo_ in ((0, wo0), (1, wo1)):
                                nc.tensor.matmul(
                                    o_ps2[:, j2, :],
                                    avT2[:, j2, 128 * cb:128 * (cb + 1)],
                                    wo_[:], start=(cb == 0), stop=(cb == 1))
                        nc.scalar.activation(out=ot4[:, 2 * p:2 * p + 2, :],
                                             in_=o_ps2[:], func=AF.Copy)
                    dst = o_d[ds(rq_of(iv), 512)].rearrange("(j p) c -> p j c",
                                                            j=TPB)
                    nc.sync.dma_start(out=dst, in_=ot4[:])

                tc.For_i_pipelined(
                    [s0_load, s1_norm, s2_project, s3_scores, s4_attend, s5_out],
                    0, niter * reps, pool=pipool, unroll=unroll,
                    staged_num_bufs=NB, staggered_reset=True,
                    hint_engines=(mybir.EngineType.PE,
                                  mybir.EngineType.Activation,
                                  mybir.EngineType.DVE,
                                  mybir.EngineType.Pool,
                                  mybir.EngineType.SP))

    nc.compile()
    return nc


_NC_CACHE = None
LAST_RESULT = None


def _to_tiles(xc):
    # [2,H,W,C] -> [256 tiles, 128 tok, C]; tile=(b,i1,jpair), tok=win*64+g1*8+g2
    xr = xc.reshape(BPC, 8, 16, 8, 8, 2, C).transpose(0, 2, 4, 5, 1, 3, 6)
    return np.ascontiguousarray(xr).reshape(ROWS, C)


def _from_tiles(oc):
    o = oc.reshape(BPC, 16, 8, 2, 8, 8, C)
    return np.ascontiguousarray(o.transpose(0, 4, 1, 5, 2, 3, 6)).reshape(
        BPC, H, W, C)


def _in_maps(nc, x, wqk, wv, wo):
    actual = [a.memorylocations[0].name for a in nc.m.functions[0].allocations
              if getattr(a, "kind", None) == "ExternalInput"
              and "partition" not in a.memorylocations[0].name]
    remap = {}
    for want in ("x_d", "wqk_d", "wv_d", "wo_d"):
        cand = [n for n in actual if want in n]
        assert cand, f"missing input {want} among {actual}"
        remap[want] = cand[0]
    maps = []
    for c in range(NCORES):
        xs = _to_tiles(x[c * BPC:(c + 1) * BPC])
        maps.append({remap["x_d"]: xs, remap["wqk_d"]: wqk,
                     remap["wv_d"]: wv, remap["wo_d"]: wo})
    return maps


def kernel(x, ln_w, ln_b, in_proj_w, in_proj_b, out_proj_w, out_proj_b, gamma):
    x = np.asarray(x, dtype=np.float32)
    ln_w = np.asarray(ln_w, np.float32); ln_b = np.asarray(ln_b, np.float32)
    in_proj_w = np.asarray(in_proj_w, np.float32)
    in_proj_b = np.asarray(in_proj_b, np.float32)
    out_proj_w = np.asarray(out_proj_w, np.float32)
    out_proj_b = np.asarray(out_proj_b, np.float32)
    gamma = np.asarray(gamma, np.float32)
    try:
        import ml_dtypes
        from concourse.bass_utils import run_bass_kernel_spmd

        bf = ml_dtypes.bfloat16
        # host-side weight folding (LN weight into W, gamma into Wo)
        wf = in_proj_w * ln_w[None, :]
        wqk = np.ascontiguousarray(wf[:2 * C].T).astype(bf)          # [C, 512]
        wv = np.ascontiguousarray(wf[2 * C:].T).astype(bf)           # [C, 256]
        wo = np.ascontiguousarray((out_proj_w * gamma[:, None]).T).astype(bf)
        # biases/ln_b are zero in this problem; bail to numpy if not
        if (np.any(ln_b) or np.any(in_proj_b) or np.any(out_proj_b)):
            raise RuntimeError("nonzero biases not supported on device path")

        global _NC_CACHE
        if _NC_CACHE is None:
            _NC_CACHE = _build_bass()
        nc = _NC_CACHE

        in_maps = _in_maps(nc, x, wqk, wv, wo)
        res = run_bass_kernel_spmd(nc, in_maps, core_ids=list(range(NCORES)))
        global LAST_RESULT
        LAST_RESULT = res
        outs = []
        for c in range(NCORES):
            od = res.results[c]
            oname = [k for k in od if "o_d" in k][0]
            outs.append(_from_tiles(od[oname]))
        return np.concatenate(outs, axis=0)
    except Exception as e:  # pragma: no cover - device fallback
        import traceback
        traceback.print_exc()
        print(f"[kernel] device path failed ({e!r}); falling back to numpy")
        return _numpy_reference(x, ln_w, ln_b, in_proj_w, in_proj_b,
                                out_proj_w, out_proj_b, gamma)


def measure_exec_ns(reps_hi=5, calls=4):
    """Estimate per-run device time by differencing reps=1 vs reps=reps_hi
    NEFF executions (constant dispatch overhead cancels)."""
    import time
    import ml_dtypes
    from concourse.bass_utils import run_bass_kernel_spmd
    bf = ml_dtypes.bfloat16
    rng = np.random.default_rng(0)
    xs = rng.standard_normal((ROWS, C), dtype=np.float32)
    wqk = (rng.standard_normal((C, 2 * C), dtype=np.float32) * 0.06).astype(bf)
    wv = (rng.standard_normal((C, C), dtype=np.float32) * 0.06).astype(bf)
    wo = (rng.standard_normal((C, C), dtype=np.float32) * 0.06).astype(bf)
    times = {}
    for reps in (1, reps_hi):
        nc = _build_bass(reps) if reps != 1 or _NC_CACHE is None else _NC_CACHE
        actual = [a.memorylocations[0].name
                  for a in nc.m.functions[0].allocations
                  if getattr(a, "kind", None) == "ExternalInput"
                  and "partition" not in a.memorylocations[0].name]
        remap = {w: [n for n in actual if w in n][0]
                 for w in ("x_d", "wqk_d", "wv_d", "wo_d")}
        im = {remap["x_d"]: xs, remap["wqk_d"]: wqk, remap["wv_d"]: wv,
              remap["wo_d"]: wo}
        in_maps = [im] * NCORES
        best = None
        for i in range(calls):
            t0 = time.time()
            run_bass_kernel_spmd(nc, in_maps, core_ids=list(range(NCORES)))
            dt = time.time() - t0
            if i > 0:  # first call pays jit/neff setup
                best = dt if best is None else min(best, dt)
        times[reps] = best
        print(f"reps={reps}: best wall {best*1e3:.1f} ms")
    ns = (times[reps_hi] - times[1]) / (reps_hi - 1) * 1e9
    return ns
